# revision 1
# baseline (speedup 1.0000x reference)
"""Multi-head self-attention (B=2, S=2048, D=1024, H=16, causal) on 8 NeuronCores.

Sharding: core c = 4*b + g handles batch b and heads 4g..4g+3 (batch x
head-group parallel). Per core:
  - q/k projections in transposed layout  qT/kT [dh, s]  (dh on partitions)
  - v projection in natural layout [s, dh] with a fused ones-column per head
    (gives the softmax denominator for free during the AV matmul)
  - causal attention in scoresT [j, i] orientation: PE scores -> ACT exp
    (scale=1/8, no max subtraction; scores ~ N(0,1) so exp never overflows)
    -> DVE causal mask multiply on diagonal blocks -> PE AV accumulate
  - normalization of attnT by the per-query denominator via a PE ones-
    broadcast + DVE multiply during PSUM eviction
  - partial o-projection out_c = merged_c @ Wo[:, cols_c].T
Host sums the 4 partial outputs per batch (the only cross-core reduction).

All matmuls run in float32r (full-rate fp32 on the TRN2 PE).
"""

import numpy as np

import concourse.bass as bass
from concourse import bacc
import concourse.mybir as mybir
import concourse.tile as tile
from concourse import bass_utils

F32 = mybir.dt.float32
F32R = mybir.dt.float32r
EXP = mybir.ActivationFunctionType.Exp

B, S, D = 2, 2048, 1024
H, DH = 16, 64
NCORES = 8
HPG = 4                  # heads per group (per core)
M = HPG * DH             # 256 per-core head dims
DC = D // 128            # 8 contraction chunks for projections
IC = 512                 # i (query) chunk for attention
JC = 128                 # j (key) chunk for attention
SCALE = 1.0 / np.sqrt(DH)


def _build_nc():
    nc = bacc.Bacc("TRN2", target_bir_lowering=False, debug=False)

    xT_d = nc.dram_tensor("xT", [D, S], F32R, kind="ExternalInput").ap()
    wqkv_d = nc.dram_tensor("wqkvT", [D, 3 * M], F32R, kind="ExternalInput").ap()
    woT_d = nc.dram_tensor("woT", [M, D], F32R, kind="ExternalInput").ap()
    mask_d = nc.dram_tensor("mask", [JC, 1536], F32, kind="ExternalInput").ap()
    onesa_d = nc.dram_tensor("ones_a", [1, 64], F32R, kind="ExternalInput").ap()
    onesb_d = nc.dram_tensor("ones_b", [JC, HPG], F32R, kind="ExternalInput").ap()
    out_d = nc.dram_tensor("out", [S, D], F32, kind="ExternalOutput").ap()

    with tile.TileContext(nc) as tc:
        _body(tc, xT_d, wqkv_d, woT_d, mask_d, onesa_d, onesb_d, out_d)
    nc.compile()
    return nc


def _body(tc, xT_d, wqkv_d, woT_d, mask_d, onesa_d, onesb_d, out_d):
    nc = tc.nc
    from contextlib import ExitStack
    ctx = ExitStack()
    with ctx:
        p_x = ctx.enter_context(tc.tile_pool(name="x", bufs=DC))
        p_w = ctx.enter_context(tc.tile_pool(name="w", bufs=DC))
        p_wo = ctx.enter_context(tc.tile_pool(name="wo", bufs=2))
        p_qk = ctx.enter_context(tc.tile_pool(name="qk", bufs=2))
        p_v = ctx.enter_context(tc.tile_pool(name="v", bufs=S // JC))
        p_mg = ctx.enter_context(tc.tile_pool(name="mg", bufs=2))
        p_probs = ctx.enter_context(tc.tile_pool(name="probs", bufs=5))
        p_small = ctx.enter_context(tc.tile_pool(name="small", bufs=1))
        p_mask = ctx.enter_context(tc.tile_pool(name="mask", bufs=1))
        p_ostg = ctx.enter_context(tc.tile_pool(name="ostg", bufs=2))
        p_ones = ctx.enter_context(tc.tile_pool(name="ones", bufs=1))

        ps_big = ctx.enter_context(tc.tile_pool(name="psb", bufs=2, space="PSUM"))
        ps_sc = ctx.enter_context(tc.tile_pool(name="pss", bufs=2, space="PSUM"))
        ps_at = ctx.enter_context(tc.tile_pool(name="psa", bufs=2, space="PSUM"))

        # ---- HAM pre-warm: the PE idles ~15us waiting for the first x/w
        # tiles anyway; a burst of discarded fp32 matmuls keeps the activity
        # monitor busy so the clock gate is at full rate (and phase-aligned)
        # when the real projections start.
        wrm = p_ones.tile([128, 512], F32, tag="warm")
        nc.vector.memset(wrm[:], 1.0)
        wrm_ps = ps_at.tile([128, 512], F32, tag="attn", name="warmps")
        for r in range(9):
            nc.tensor.matmul(wrm_ps[:], wrm[:, 0:128], wrm[:],
                             start=(r == 0), stop=(r == 8))
        nc.scalar.copy(wrm[:, 0:1], wrm_ps[:, 0:1])  # keep alive vs DCE

        # ---- input loads: combined qkv weight tile + x tile interleaved in
        # the order the projection matmuls consume them (dc ascending), so
        # the first matmul can start ~5us in instead of after the full load.
        w_t, x_t = [], []
        for dc in range(DC):
            wt = p_w.tile([128, 3 * M], F32R, tag="w")
            nc.sync.dma_start(wt[:], wqkv_d[dc * 128:(dc + 1) * 128, :])
            w_t.append(wt)
            xt = p_x.tile([128, S], F32R, tag="x")
            nc.sync.dma_start(xt[:], xT_d[dc * 128:(dc + 1) * 128, :])
            x_t.append(xt)
        wo_t = []
        for kc in range(2):
            t = p_wo.tile([128, D], F32R, tag="wo")
            nc.sync.dma_start(t[:], woT_d[kc * 128:(kc + 1) * 128, :])
            wo_t.append(t)
        mask_t = p_mask.tile([JC, 1536], F32, tag="mask")
        nc.sync.dma_start(mask_t[:], mask_d[:])
        ones_t = p_ones.tile([1, 64], F32R, tag="ones")
        nc.sync.dma_start(ones_t[:], onesa_d[:])
        onesb_t = p_ones.tile([JC, HPG], F32R, tag="onesb")
        nc.sync.dma_start(onesb_t[:], onesb_d[:])

        # ---- projection building blocks ----
        q_t, k_t = {}, {}

        def qk_proj(mc):
            # qT/kT [m, s] = sum_d WT[d, m] xT[d, s], m-chunk mc.
            # The d-contraction is split in two half-groups (dc 0-3, dc 4-7)
            # merged at eviction, so the in-order PE never stalls waiting for
            # the last x tiles to arrive from HBM.
            for woff, store, tg in ((0, q_t, "qT"), (M, k_t, "kT")):
                dst = p_qk.tile([128, S], F32R, tag=tg, name=f"{tg}{mc}")
                for s4 in range(S // 512):
                    sl = slice(s4 * 512, (s4 + 1) * 512)
                    psa = ps_big.tile([128, 512], F32, tag="proj")
                    for dc in range(DC // 2):
                        nc.tensor.matmul(
                            psa[:],
                            w_t[dc][:, woff + mc * 128:woff + (mc + 1) * 128],
                            x_t[dc][:, sl],
                            start=(dc == 0), stop=(dc == DC // 2 - 1))
                    psb = ps_sc.tile([128, 2 * IC], F32, tag="scores")
                    for dc in range(DC // 2, DC):
                        nc.tensor.matmul(
                            psb[:, 0:512],
                            w_t[dc][:, woff + mc * 128:woff + (mc + 1) * 128],
                            x_t[dc][:, sl],
                            start=(dc == DC // 2), stop=(dc == DC - 1))
                    nc.scalar.copy(dst[:, sl], psa[:])
                    nc.vector.tensor_add(dst[:, sl], dst[:, sl], psb[:, 0:512])
                store[mc] = dst

        v_t = {}

        def v_proj(sc):
            # v[s, m] tile for j-chunk sc: per head h cols h*65..h*65+63 = v,
            # col h*65+64 = 1.0 (softmax denominator column)
            vt = p_v.tile([JC, HPG * (DH + 1)], F32R, tag="v", name=f"v{sc}")
            nc.vector.tensor_copy(
                vt[:].rearrange("p (h e) -> p h e", h=HPG)[:, :, DH:DH + 1].squeeze(2),
                onesb_t[:])
            psa = ps_big.tile([128, 512], F32, tag="proj")
            for dc in range(DC // 2):
                nc.tensor.matmul(
                    psa[:, 0:M],
                    x_t[dc][:, sc * 128:(sc + 1) * 128],
                    w_t[dc][:, 2 * M:3 * M],
                    start=(dc == 0), stop=(dc == DC // 2 - 1))
            psb = ps_sc.tile([128, 2 * IC], F32, tag="scores")
            for dc in range(DC // 2, DC):
                nc.tensor.matmul(
                    psb[:, 0:M],
                    x_t[dc][:, sc * 128:(sc + 1) * 128],
                    w_t[dc][:, 2 * M:3 * M],
                    start=(dc == DC // 2), stop=(dc == DC - 1))
            nc.scalar.activation(
                vt[:].rearrange("p (h e) -> p h e", h=HPG)[:, :, 0:DH],
                psa[:, 0:M].rearrange("p (h d) -> p h d", h=HPG),
                mybir.ActivationFunctionType.Copy)
            src_b = psb[:, 0:M].rearrange("p (h d) -> p h d", h=HPG)
            dst = vt[:].rearrange("p (h e) -> p h e", h=HPG)[:, :, 0:DH]
            nc.vector.tensor_add(dst, dst, src_b)
            v_t[sc] = vt

        # ---- attention, scoresT orientation ----
        # Emission is software-pipelined: the normalize/evict of a group
        # (reciprocal -> PE ones-broadcast -> DVE mul) is emitted one group
        # later so the reciprocal never stalls the in-order PE stream.
        # o-projection blocks are emitted as soon as their i-range has all
        # 4 heads normalized.
        mg_t = [p_mg.tile([128, S], F32R, tag="mgT", name=f"mg{i}")
                for i in range(M // 128)]

        def attend(h, ic):
            # j-chunks processed in PAIRS sharing a [128, 2*IC] PSUM tile and
            # a single exp instruction (halves ACT instruction count). Within
            # a sub-block of 4 pairs: all scores first, then all AVs, so the
            # in-order PE stream never waits on ACT latency.
            qk_tile = h // 2
            prow = 64 * (h % 2)
            njc = (ic * IC) // JC + IC // JC  # causal: j chunks 0..njc-1
            at_ps = ps_at.tile([DH + 1, IC], F32, tag="attn")
            pairs = [(p, min(p + 2, njc)) for p in range(0, njc, 2)]
            SUBP = 4
            for p0 in range(0, len(pairs), SUBP):
                blk = pairs[p0:p0 + SUBP]
                prs = []
                for (ja, jb) in blk:
                    sc_ps = ps_sc.tile([128, 2 * IC], F32, tag="scores")
                    pr = p_probs.tile([JC, 2 * IC], F32R, tag="probs")
                    for u, jc in enumerate(range(ja, jb)):
                        nc.tensor.matmul(
                            sc_ps[:, u * IC:(u + 1) * IC],
                            k_t[qk_tile][prow:prow + DH, jc * JC:(jc + 1) * JC],
                            q_t[qk_tile][prow:prow + DH, ic * IC:(ic + 1) * IC],
                            start=True, stop=True)
                    nc.scalar.activation(pr[:], sc_ps[:], EXP, scale=SCALE)
                    # The last 4 j-chunks of a group are diagonal, always as
                    # two pairs with delta=(0,128) and (256,384). Each pair is
                    # masked by ONE strided multiply against a host mask laid
                    # out to cover both chunks' strips + zero regions.
                    delta0 = ja * JC - ic * IC
                    if delta0 == 0:        # pair A: deltas 0 and 128
                        pv = pr[:].rearrange("p (a b) -> p a b", a=2)[:, :, 0:256]
                        nc.vector.tensor_mul(
                            pv, pv,
                            mask_t[:, 0:512].rearrange("p (a b) -> p a b", a=2))
                    elif delta0 == 256:    # pair B: deltas 256 and 384
                        nc.vector.tensor_mul(
                            pr[:], pr[:], mask_t[:, 512:1536])
                    prs.append(pr)
                for (ja, jb), pr in zip(blk, prs):
                    for u, jc in enumerate(range(ja, jb)):
                        nc.tensor.matmul(
                            at_ps[:],
                            v_t[jc][:, h * (DH + 1):(h + 1) * (DH + 1)],
                            pr[:, u * IC:(u + 1) * IC],
                            start=(jc == 0), stop=(jc == njc - 1))
            return at_ps

        def normalize(h, ic, at_ps):
            # rows 0..63 / row 64 (denominator), evicted into mergedT
            qk_tile = h // 2
            prow = 64 * (h % 2)
            den = p_small.tile([1, IC], F32, tag="den")
            nc.vector.tensor_copy(den[:], at_ps[DH:DH + 1, :])
            rc32 = p_small.tile([1, IC], F32, tag="recip32")
            nc.vector.reciprocal_approx_fast(rc32[:], den[:])
            rc = p_small.tile([1, IC], F32R, tag="recip")
            nc.vector.tensor_copy(rc[:], rc32[:])
            bc_ps = ps_big.tile([DH, IC], F32, tag="proj")
            nc.tensor.matmul(bc_ps[:], ones_t[:], rc[:], start=True, stop=True)
            bc_sb = p_small.tile([DH, IC], F32, tag="bcast")
            nc.vector.tensor_copy(bc_sb[:], bc_ps[:])
            nc.vector.tensor_mul(
                mg_t[qk_tile][prow:prow + DH, ic * IC:(ic + 1) * IC],
                at_ps[0:DH, :], bc_sb[:])

        def oproj(sc):
            # out[s, o] = sum_k mergedT[k, s] woT[k, o] for s-chunk sc.
            # The two half-evictions are split across DVE and ACT to keep
            # either engine from becoming the attention-phase bottleneck.
            stg = p_ostg.tile([128, D], F32, tag="ostg")
            for nn in range(2):
                ps = ps_big.tile([128, 512], F32, tag="proj")
                for kc in range(2):
                    nc.tensor.matmul(
                        ps[:],
                        mg_t[kc][:, sc * 128:(sc + 1) * 128],
                        wo_t[kc][:, nn * 512:(nn + 1) * 512],
                        start=(kc == 0), stop=(kc == 1))
                if nn == 0:
                    nc.vector.tensor_copy(stg[:, 0:512], ps[:])
                else:
                    nc.scalar.copy(stg[:, 512:1024], ps[:])
            nc.sync.dma_start(out_d[sc * 128:(sc + 1) * 128, :], stg[:])

        # ---- interleaved schedule ----
        # Attention groups (ascending ic) are woven between projection blocks
        # so ACT's exp stream overlaps the PE-dense projection phase, and the
        # normalize/o-proj of a group is emitted one group later so neither
        # the reciprocal chain nor the mergedT eviction gates the in-order PE
        # stream.
        sched = [
            ("qk", 0), ("v", 0, 4),
            ("a", 0, 0), ("a", 1, 0),
            ("qk", 1),
            ("a", 2, 0), ("a", 3, 0),
            ("v", 4, 8),
            ("a", 0, 1), ("a", 1, 1), ("a", 2, 1), ("a", 3, 1),
            ("v", 8, 12),
            ("a", 0, 2), ("a", 1, 2), ("a", 2, 2), ("a", 3, 2),
            ("v", 12, 16),
            ("a", 0, 3), ("a", 1, 3), ("a", 2, 3), ("a", 3, 3),
        ]
        pending = None
        pending_oproj = []
        for item in sched:
            if item[0] == "qk":
                qk_proj(item[1])
                continue
            if item[0] == "v":
                for sc in range(item[1], item[2]):
                    v_proj(sc)
                continue
            _, h, ic = item
            at = attend(h, ic)
            for sc in pending_oproj:
                oproj(sc)
            pending_oproj = []
            if pending is not None:
                normalize(*pending)
                if pending[0] == HPG - 1:  # last head of its ic: mergedT done
                    pending_oproj = list(range(4 * pending[1], 4 * pending[1] + 4))
            pending = (h, ic, at)
        normalize(*pending)
        for sc in pending_oproj + list(range(4 * pending[1], 4 * pending[1] + 4)):
            oproj(sc)


_NC_CACHE = None


def _get_nc():
    global _NC_CACHE
    if _NC_CACHE is None:
        _NC_CACHE = _build_nc()
    return _NC_CACHE


def _causal_mask_tile():
    # BIG[j, c] = 1.0 if j <= c - 384 else 0.0 (zeros | 128-wide triangle | ones).
    # Layout [128, 1536]: cols 0:512  = concat(BIG[384:640], BIG[256:512]) --
    # masks a (delta=0, delta=128) chunk pair in one strided multiply;
    # cols 512:1536 = concat(BIG[128:640], BIG[0:512]) -- masks a
    # (delta=256, delta=384) pair as one full-width multiply.
    j = np.arange(JC)[:, None]
    c = np.arange(896)[None, :]
    big = (j <= c - 384).astype(np.float32)
    return np.concatenate(
        [big[:, 384:640], big[:, 256:512], big[:, 128:640], big[:, 0:512]],
        axis=1)


def _prepare_in_maps(inputs):
    x = np.asarray(inputs["in_features"], dtype=np.float32)
    wqT = np.ascontiguousarray(np.asarray(inputs["q_proj_weight"], np.float32).T)
    wkT = np.ascontiguousarray(np.asarray(inputs["k_proj_weight"], np.float32).T)
    wvT = np.ascontiguousarray(np.asarray(inputs["v_proj_weight"], np.float32).T)
    woT = np.ascontiguousarray(np.asarray(inputs["o_proj_weight"], np.float32).T)
    xT = [np.ascontiguousarray(x[b].T) for b in range(B)]
    mask = _causal_mask_tile()

    in_maps = []
    for c in range(NCORES):
        b, g = divmod(c, HPG)
        ms = slice(g * M, (g + 1) * M)
        in_maps.append({
            "xT": xT[b],
            "wqkvT": np.ascontiguousarray(
                np.concatenate([wqT[:, ms], wkT[:, ms], wvT[:, ms]], axis=1)),
            "woT": np.ascontiguousarray(woT[ms, :]),
            "mask": mask,
            "ones_a": np.ones((1, 64), np.float32),
            "ones_b": np.ones((JC, HPG), np.float32),
        })
    return in_maps


def kernel(q_proj_weight, k_proj_weight, v_proj_weight, o_proj_weight, in_features):
    in_dtype = np.asarray(in_features).dtype
    in_maps = _prepare_in_maps({
        "q_proj_weight": q_proj_weight,
        "k_proj_weight": k_proj_weight,
        "v_proj_weight": v_proj_weight,
        "o_proj_weight": o_proj_weight,
        "in_features": in_features,
    })
    nc = _get_nc()
    res = bass_utils.run_bass_kernel_spmd(nc, in_maps, core_ids=list(range(NCORES)))
    out = np.zeros((B, S, D), dtype=np.float32)
    for c in range(NCORES):
        out[c // HPG] += res.results[c]["out"]
    return out.astype(in_dtype)



# revision 8
# speedup vs baseline: 1.4228x; 1.4228x over previous
"""Multi-head self-attention (B=2, S=2048, D=1024, H=16, causal) on 8 NeuronCores.

Sharding: core c = 4*b + g handles batch b and heads 4g..4g+3 (batch x
head-group parallel). Per core (all matmul operands bf16, fp32 PSUM accum):
  - q/k projections in transposed layout  qT/kT [dh, s]  (dh on partitions)
  - v projection in natural layout [s, dh] with a fused ones-column per head
    (gives the softmax denominator for free during the AV matmul)
  - causal attention in scoresT [j, i] orientation, single-chunk [128, 512]
    score tiles: PE scores -> ACT exp (scale=1/8, no max subtraction) ->
    DVE causal mask multiply on diagonal chunks -> PE AV accumulate.
    Emission is software-pipelined at CHUNK granularity: the AV stream of
    step g-1 and projection/o-proj blocks are zipped between the score
    matmuls of step g, so the in-order PE never waits for ACT's exp stream
    and the HAM clock gate stays released.
  - normalization of attnT by the per-query denominator via DVE reciprocal
    -> GPSIMD partition_broadcast -> DVE multiply into mergedT (bf16)
  - partial o-projection out_c = merged_c @ Wo[:, cols_c].T, bf16 staging
Host sums the 4 bf16 partial outputs per batch in f32 (the only cross-core
reduction).

bf16 matmuls run at 1 cycle/row on the TRN2 PE (fp32 modes are 2x slower)
and enable the compiler's fast-weight-load path for 128-col stationaries.
"""

import numpy as np
import ml_dtypes

import concourse.bass as bass
from concourse import bacc
import concourse.mybir as mybir
import concourse.tile as tile
from concourse import bass_utils

F32 = mybir.dt.float32
BF16 = mybir.dt.bfloat16
EXP = mybir.ActivationFunctionType.Exp
BF = ml_dtypes.bfloat16

B, S, D = 2, 2048, 1024
H, DH = 16, 64
NCORES = 8
HPG = 4                  # heads per group (per core)
M = HPG * DH             # 256 per-core head dims
DC = D // 128            # 8 contraction chunks for projections
IC = 512                 # i (query) chunk for attention
JC = 128                 # j (key) chunk for attention
SCALE = 1.0 / np.sqrt(DH)


def _build_nc():
    nc = bacc.Bacc("TRN2", target_bir_lowering=False, debug=False)

    xT_d = nc.dram_tensor("xT", [D, S], BF16, kind="ExternalInput").ap()
    wqkv_d = nc.dram_tensor("wqkvT", [D, 3 * M], BF16, kind="ExternalInput").ap()
    woT_d = nc.dram_tensor("woT", [M, D], BF16, kind="ExternalInput").ap()
    mask_d = nc.dram_tensor("mask", [JC, 4 * IC], BF16, kind="ExternalInput").ap()
    onesb_d = nc.dram_tensor("ones_b", [JC, HPG], BF16, kind="ExternalInput").ap()
    out_d = nc.dram_tensor("out", [S, D], BF16, kind="ExternalOutput").ap()

    with tile.TileContext(nc) as tc:
        _body(tc, xT_d, wqkv_d, woT_d, mask_d, onesb_d, out_d)
    nc.compile()
    return nc


def _body(tc, xT_d, wqkv_d, woT_d, mask_d, onesb_d, out_d):
    nc = tc.nc
    from contextlib import ExitStack
    ctx = ExitStack()
    with ctx:
        p_x = ctx.enter_context(tc.tile_pool(name="x", bufs=DC))
        p_w = ctx.enter_context(tc.tile_pool(name="w", bufs=DC))
        p_wo = ctx.enter_context(tc.tile_pool(name="wo", bufs=2))
        p_qk = ctx.enter_context(tc.tile_pool(name="qk", bufs=4))
        p_v = ctx.enter_context(tc.tile_pool(name="v", bufs=S // JC))
        p_mg = ctx.enter_context(tc.tile_pool(name="mg", bufs=2))
        p_probs = ctx.enter_context(tc.tile_pool(name="probs", bufs=36))
        p_small = ctx.enter_context(tc.tile_pool(name="small", bufs=4))
        p_bc = ctx.enter_context(tc.tile_pool(name="bc", bufs=4))
        p_mask = ctx.enter_context(tc.tile_pool(name="mask", bufs=1))
        p_ostg = ctx.enter_context(tc.tile_pool(name="ostg", bufs=2))
        p_ones = ctx.enter_context(tc.tile_pool(name="ones", bufs=1))

        ps_at = ctx.enter_context(tc.tile_pool(name="psa", bufs=3, space="PSUM"))
        ps_sc = ctx.enter_context(tc.tile_pool(name="pss", bufs=3, space="PSUM"))
        ps_pr = ctx.enter_context(tc.tile_pool(name="psp", bufs=2, space="PSUM"))

        # ---- HAM pre-warm: a burst of discarded matmuls while the first
        # x/w tiles stream in keeps the PE activity monitor busy so the
        # clock gate is released when the real projections start.
        wrm = p_ones.tile([128, 512], BF16, tag="warm")
        nc.vector.memset(wrm[:], 1.0)
        ones_a = p_ones.tile([1, DH], BF16, tag="onesa")
        nc.vector.memset(ones_a[:], 1.0)
        wrm_ps = ps_at.tile([128, 512], F32, tag="attn", name="warmps")
        for r in range(9):
            nc.tensor.matmul(wrm_ps[:], wrm[:, 0:128], wrm[:],
                             start=(r == 0), stop=(r == 8))
        nc.scalar.copy(wrm[:, 0:1], wrm_ps[:, 0:1])  # keep alive vs DCE

        # ---- input loads, in the order the projection matmuls consume them
        w_t, x_t = [], []
        for dc in range(DC):
            wt = p_w.tile([128, 3 * M], BF16, tag="w")
            nc.sync.dma_start(wt[:], wqkv_d[dc * 128:(dc + 1) * 128, :])
            w_t.append(wt)
            xt = p_x.tile([128, S], BF16, tag="x")
            nc.sync.dma_start(xt[:], xT_d[dc * 128:(dc + 1) * 128, :])
            x_t.append(xt)
        wo_t = []
        for kc in range(2):
            t = p_wo.tile([128, D], BF16, tag="wo")
            nc.sync.dma_start(t[:], woT_d[kc * 128:(kc + 1) * 128, :])
            wo_t.append(t)
        mask_t = p_mask.tile([JC, 4 * IC], BF16, tag="mask")
        nc.sync.dma_start(mask_t[:], mask_d[:])
        onesb_t = p_ones.tile([JC, HPG], BF16, tag="onesb")
        nc.sync.dma_start(onesb_t[:], onesb_d[:])

        # ---- projection building blocks ----
        q_t = {mc: p_qk.tile([128, S], BF16, tag="qk", name=f"qT{mc}")
               for mc in range(2)}
        k_t = {mc: p_qk.tile([128, S], BF16, tag="qk", name=f"kT{mc}")
               for mc in range(2)}
        mg_t = [p_mg.tile([128, S], BF16, tag="mgT", name=f"mg{i}")
                for i in range(M // 128)]

        def qk_block(tg, mc, s4, split):
            # qT/kT [m, s] = sum_d WT[d, m] xT[d, s]; m-chunk mc, s-chunk s4.
            # split=True merges two half-contractions at eviction so the
            # in-order PE never stalls on the last x tiles during the DMA
            # ramp at kernel start.
            woff = 0 if tg == "q" else M
            dst = (q_t if tg == "q" else k_t)[mc]
            sl = slice(s4 * 512, (s4 + 1) * 512)
            wsl = slice(woff + mc * 128, woff + (mc + 1) * 128)
            if split:
                psa = ps_pr.tile([128, 512], F32, tag="proj")
                for dc in range(DC // 2):
                    nc.tensor.matmul(psa[:], w_t[dc][:, wsl], x_t[dc][:, sl],
                                     start=(dc == 0), stop=(dc == DC // 2 - 1))
                psb = ps_pr.tile([128, 512], F32, tag="proj")
                for dc in range(DC // 2, DC):
                    nc.tensor.matmul(psb[:], w_t[dc][:, wsl], x_t[dc][:, sl],
                                     start=(dc == DC // 2), stop=(dc == DC - 1))
                nc.vector.tensor_copy(dst[:, sl], psa[:])
                nc.vector.tensor_add(dst[:, sl], dst[:, sl], psb[:])
            else:
                ps = ps_pr.tile([128, 512], F32, tag="proj")
                for dc in range(DC):
                    nc.tensor.matmul(ps[:], w_t[dc][:, wsl], x_t[dc][:, sl],
                                     start=(dc == 0), stop=(dc == DC - 1))
                nc.vector.tensor_copy(dst[:, sl], ps[:])

        v_t = {}

        def v_block(sc, split):
            # v[s, m] tile for j-chunk sc: per head h cols h*65..h*65+63 = v,
            # col h*65+64 = 1.0 (softmax denominator column)
            vt = p_v.tile([JC, HPG * (DH + 1)], BF16, tag="v", name=f"v{sc}")
            vv = vt[:].rearrange("p (h e) -> p h e", h=HPG)
            nc.vector.tensor_copy(vv[:, :, DH:DH + 1].squeeze(2), onesb_t[:])
            xsl = slice(sc * 128, (sc + 1) * 128)
            if split:
                psa = ps_pr.tile([128, 512], F32, tag="proj")
                for dc in range(DC // 2):
                    nc.tensor.matmul(psa[:, 0:M], x_t[dc][:, xsl],
                                     w_t[dc][:, 2 * M:3 * M],
                                     start=(dc == 0), stop=(dc == DC // 2 - 1))
                psb = ps_pr.tile([128, 512], F32, tag="proj")
                for dc in range(DC // 2, DC):
                    nc.tensor.matmul(psb[:, 0:M], x_t[dc][:, xsl],
                                     w_t[dc][:, 2 * M:3 * M],
                                     start=(dc == DC // 2), stop=(dc == DC - 1))
                nc.scalar.activation(
                    vv[:, :, 0:DH],
                    psa[:, 0:M].rearrange("p (h d) -> p h d", h=HPG),
                    mybir.ActivationFunctionType.Copy)
                nc.vector.tensor_add(
                    vv[:, :, 0:DH], vv[:, :, 0:DH],
                    psb[:, 0:M].rearrange("p (h d) -> p h d", h=HPG))
            else:
                ps = ps_pr.tile([128, 512], F32, tag="proj")
                for dc in range(DC):
                    nc.tensor.matmul(ps[:, 0:M], x_t[dc][:, xsl],
                                     w_t[dc][:, 2 * M:3 * M],
                                     start=(dc == 0), stop=(dc == DC - 1))
                nc.scalar.activation(
                    vv[:, :, 0:DH],
                    ps[:, 0:M].rearrange("p (h d) -> p h d", h=HPG),
                    mybir.ActivationFunctionType.Copy)
            v_t[sc] = vt

        def oproj_block(sc):
            # out[s, o] = sum_k mergedT[k, s] woT[k, o] for s-chunk sc.
            # Evictions split across DVE and ACT.
            stg = p_ostg.tile([128, D], BF16, tag="ostg")
            for nn in range(2):
                ps = ps_pr.tile([128, 512], F32, tag="proj")
                for kc in range(2):
                    nc.tensor.matmul(
                        ps[:], mg_t[kc][:, sc * 128:(sc + 1) * 128],
                        wo_t[kc][:, nn * 512:(nn + 1) * 512],
                        start=(kc == 0), stop=(kc == 1))
                if nn == 0:
                    nc.vector.tensor_copy(stg[:, 0:512], ps[:])
                else:
                    nc.scalar.copy(stg[:, 512:1024], ps[:])
            nc.sync.dma_start(out_d[sc * 128:(sc + 1) * 128, :], stg[:])

        # ---- attention ----
        probs_store = {}   # (h, ic) -> list of probs tiles
        at_store = {}      # (h, ic) -> at_ps handle

        def score_chunk(h, ic, jc):
            qk_tile, prow = h // 2, 64 * (h % 2)
            sc_ps = ps_sc.tile([JC, IC], F32, tag="sc")
            pr = p_probs.tile([JC, IC], BF16, tag="probs")
            nc.tensor.matmul(
                sc_ps[:],
                k_t[qk_tile][prow:prow + DH, jc * JC:(jc + 1) * JC],
                q_t[qk_tile][prow:prow + DH, ic * IC:(ic + 1) * IC],
                start=True, stop=True)
            nc.scalar.activation(pr[:], sc_ps[:], EXP, scale=SCALE)
            d = jc - 4 * ic
            if d >= 0:  # diagonal chunk: causal mask (i >= j + 128*d keeps)
                nc.vector.tensor_mul(pr[:], pr[:],
                                     mask_t[:, d * IC:(d + 1) * IC])
            probs_store[(h, ic)].append(pr)

        def av_chunk(h, ic, jc, njc):
            if jc == 0:
                at_store[(h, ic)] = ps_at.tile([DH + 1, IC], F32, tag="attn",
                                               name=f"at{h}_{ic}")
            nc.tensor.matmul(
                at_store[(h, ic)][:],
                v_t[jc][:, h * (DH + 1):(h + 1) * (DH + 1)],
                probs_store[(h, ic)][jc][:],
                start=(jc == 0), stop=(jc == njc - 1))

        def normalize_pair(ic, pairidx):
            # heads (2*pairidx, 2*pairidx+1): rows 0..63 / 64..127 of
            # mergedT tile `pairidx`, columns ic*512..  DVE reciprocal of the
            # fused denominator row -> GPSIMD broadcast -> DVE multiply.
            h0 = 2 * pairidx
            at0, at1 = at_store.pop((h0, ic)), at_store.pop((h0 + 1, ic))
            isl = slice(ic * IC, (ic + 1) * IC)
            for u, at in ((0, at0), (1, at1)):
                den = p_small.tile([1, IC], F32, tag="den")
                nc.vector.tensor_copy(den[:], at[DH:DH + 1, :])
                rc32 = p_small.tile([1, IC], F32, tag="rc32")
                nc.vector.reciprocal_approx_fast(rc32[:], den[:])
                rcb = p_small.tile([1, IC], BF16, tag="rcb")
                nc.vector.tensor_copy(rcb[:], rc32[:])
                bc_ps = ps_sc.tile([DH, IC], F32, tag="sc", name=f"bc{u}_{ic}")
                nc.tensor.matmul(bc_ps[:], ones_a[:], rcb[:],
                                 start=True, stop=True)
                bc = p_bc.tile([DH, IC], BF16, tag="bc")
                nc.scalar.copy(bc[:], bc_ps[:])
                nc.vector.tensor_mul(
                    mg_t[pairidx][u * DH:(u + 1) * DH, isl],
                    at[0:DH, :], bc[:])
            # free the probs tiles of both heads
            probs_store.pop((h0, ic))
            probs_store.pop((h0 + 1, ic))

        # ---- schedule ----
        # Pre-attention: q/k for head-pair 0 (split halves for the DMA ramp),
        # first 4 v chunks (split), then q/k for head-pair 1 at full speed.
        for s4 in range(4):
            qk_block("q", 0, s4, split=True)
            qk_block("k", 0, s4, split=True)
        for sc in range(4):
            v_block(sc, split=True)
        for s4 in range(4):
            qk_block("q", 1, s4, split=False)
            qk_block("k", 1, s4, split=False)

        # Work queue of PE blocks zipped between score chunks: remaining v
        # chunks now, o-proj blocks pushed as their mergedT columns complete.
        from collections import deque
        work = deque(("v", sc) for sc in range(4, S // JC))

        def pop_work():
            if not work:
                return
            kind, arg = work.popleft()
            if kind == "v":
                v_block(arg, split=False)
            else:
                oproj_block(arg)

        steps = [(ic, h) for ic in range(S // IC) for h in range(HPG)]
        prev = None
        for g in steps:
            ic, h = g
            njc = 4 * ic + 4
            probs_store[(h, ic)] = []
            if prev is not None:
                njc_p = 4 * prev[0] + 4
            for c in range(njc):
                score_chunk(h, ic, c)
                if prev is not None and c < njc_p:
                    av_chunk(prev[1], prev[0], c, njc_p)
                if c % 4 == 1:
                    pop_work()
            if prev is not None and prev[1] % 2 == 1:
                normalize_pair(prev[0], prev[1] // 2)
                if prev[1] == HPG - 1:
                    for sc in range(4 * prev[0], 4 * prev[0] + 4):
                        work.append(("o", sc))
            prev = g

        # ---- tail flush ----
        ic, h = prev
        njc_p = 4 * ic + 4
        for c in range(njc_p):
            av_chunk(h, ic, c, njc_p)
            if c % 4 == 1:
                pop_work()
        normalize_pair(ic, h // 2)
        for sc in range(4 * ic, 4 * ic + 4):
            work.append(("o", sc))
        while work:
            pop_work()


_NC_CACHE = None


def _get_nc():
    global _NC_CACHE
    if _NC_CACHE is None:
        _NC_CACHE = _build_nc()
    return _NC_CACHE


def _causal_mask_tile():
    # mask[j, d*512 + i] = 1.0 if i >= j + 128*d else 0.0, for the four
    # diagonal-chunk offsets d in 0..3.
    j = np.arange(JC)[:, None]
    i = np.arange(IC)[None, :]
    return np.concatenate(
        [(i >= j + 128 * d).astype(np.float32) for d in range(4)],
        axis=1).astype(BF)


def _prepare_in_maps(inputs):
    x = np.asarray(inputs["in_features"], dtype=np.float32)
    wqT = np.asarray(inputs["q_proj_weight"], np.float32).T
    wkT = np.asarray(inputs["k_proj_weight"], np.float32).T
    wvT = np.asarray(inputs["v_proj_weight"], np.float32).T
    woT = np.asarray(inputs["o_proj_weight"], np.float32).T
    xT = [np.ascontiguousarray(x[b].T).astype(BF) for b in range(B)]
    mask = _causal_mask_tile()

    in_maps = []
    for c in range(NCORES):
        b, g = divmod(c, HPG)
        ms = slice(g * M, (g + 1) * M)
        in_maps.append({
            "xT": xT[b],
            "wqkvT": np.ascontiguousarray(
                np.concatenate([wqT[:, ms], wkT[:, ms], wvT[:, ms]],
                               axis=1)).astype(BF),
            "woT": np.ascontiguousarray(woT[ms, :]).astype(BF),
            "mask": mask,
            "ones_b": np.ones((JC, HPG), BF),
        })
    return in_maps


def kernel(q_proj_weight, k_proj_weight, v_proj_weight, o_proj_weight, in_features):
    in_dtype = np.asarray(in_features).dtype
    in_maps = _prepare_in_maps({
        "q_proj_weight": q_proj_weight,
        "k_proj_weight": k_proj_weight,
        "v_proj_weight": v_proj_weight,
        "o_proj_weight": o_proj_weight,
        "in_features": in_features,
    })
    nc = _get_nc()
    res = bass_utils.run_bass_kernel_spmd(nc, in_maps, core_ids=list(range(NCORES)))
    out = np.zeros((B, S, D), dtype=np.float32)
    for c in range(NCORES):
        out[c // HPG] += res.results[c]["out"].astype(np.float32)
    return out.astype(in_dtype)
